# revision 23
# baseline (speedup 1.0000x reference)
"""BarrierNet (MLP heads + dCBF closed-form QP) Trainium2 Bass kernel.

Data-parallel over 8 NeuronCores: batch 262144 is split into 8 shards of
32768 rows; the tiny MLP weights are replicated (folded with mean/std on
host) and each core computes its full shard independently. No collectives.

Per-core dataflow (feature-major matmuls, batch in the free dim, all
matmuls bf16 in / fp32 PSUM out; PE-bound):
  L1: h^T = relu(W1eff @ x^T + b1eff); two adjacent 512-col chunks of
      one group per unit, same stationary, into a [128, 1024] PSUM pair
      drained by one ScalarE activation.
  L2: mid^T = relu(Wmid @ h^T + bmid), drained chunk-wise by VectorE
      (every L2_ACT_EVERY-th by ScalarE for balance).
  L3: heads via a sliding-window weight trick, split into an A set
      (first 16 chunks of each half -> PSUM partitions 0:64) and a B set
      (last 16 -> 64:128) so each quarter of the batch drains as soon as
      its 16 chunks are accumulated (tail shrinks to one quarter's QP).
  The PE stream is software-pipelined with a 2-unit lag
  (L1(u) -> L2(u-1) -> L3(u-2)) so no matmul waits on a PSUM drain.
  QP: the host precomputes [dx, dy, wrapped sin/cos phases, 8*barrier,
      v^2] in fp16 (feature-blocked, SBUF layout built host-side), so
      qp_pre is just 2 ScalarE sins + a contiguous GpSimd
      tensor_tensor chain per half; the scalar folds + reciprocal run
      on VectorE under a tile_wait_until floor (the Tile scheduler's
      cost model underestimates GpSimd, and without the floor it queues
      them ahead of the bulk m copies, head-of-line blocking VectorE
      and starving the PE). qp_post runs per quarter on GpSimd; the
      final quarter is column-split across VectorE+GpSimd to halve the
      tail latency.

Matmul perf notes: 512 cols is the ISA max per matmul; fp32r executes
as fp32 HIGH (4x slower) on this walrus, bf16 streams 1 col/cycle.
The HAM clock duty-cycles between full and half rate under sustained
all-core load, so the sustained floor is ~(512 + ldw) cycles at
1.2 GHz per matmul; fp8 DoubleRow for L3 measured 2.0e-2 error (at the
gate) and was rejected.
"""

import os
import sys

import numpy as np

sys.path.insert(0, "/opt/trn_rl_repo")

import concourse.bass as bass
import concourse.tile as tile
from concourse import mybir
from concourse.bass_utils import run_bass_kernel_spmd

F32 = mybir.dt.float32
F32R = mybir.dt.float32r
BF16 = mybir.dt.bfloat16
FP16 = mybir.dt.float16
AF = mybir.ActivationFunctionType
ALU = mybir.AluOpType

B = 262144
NF = 8
NCORES = 8
BC = B // NCORES   # 32768 rows per core
CH = 512           # chunk columns (one PSUM bank of fp32)
GSZ = 8192         # rows per group (4 groups per core)
HB = 16384         # rows per half
QB = 8192          # rows per quarter
HQ = 64            # quarter batch-major free width (r_q = p*64 + c)
OBS_X, OBS_Y, RAD = 4.0, 6.0, 1.5
PI = float(np.pi)

N_WARMUP_MM = int(os.environ.get("KERNEL_WARMUP", "0"))
# every Nth L2 copy goes to ScalarE instead of VectorE (0 = none)
L2_ACT_EVERY = int(os.environ.get("KERNEL_L2_ACT_EVERY", "3"))

_MMDT_MAP = {"f32r": F32R, "f32": F32, "bf16": BF16}
MMDT = _MMDT_MAP[os.environ.get("KERNEL_MM_DTYPE", "bf16")]


def _build_program(std4, mean4, split_waits=True, reps=1):
    nc = bass.Bass()

    x_bm = nc.dram_tensor("x_bm", [256, 2 * HQ * NF], FP16,
                          kind="ExternalInput")
    x_tr = nc.dram_tensor("x_tr", [32, GSZ], MMDT, kind="ExternalInput")
    w1t = nc.dram_tensor("w1t", [8, 128], MMDT, kind="ExternalInput")
    wmw = nc.dram_tensor("wmw", [128, 208], MMDT, kind="ExternalInput")
    bias3 = nc.dram_tensor("bias3", [128, 3], F32, kind="ExternalInput")
    u_out = nc.dram_tensor("u", [BC, 2], F32, kind="ExternalOutput")

    with tile.TileContext(nc) as tc:
        from contextlib import ExitStack

        with ExitStack() as ctx:
            _body(ctx, tc, x_bm, x_tr, w1t, wmw, bias3, u_out,
                  std4, mean4, reps)
    if split_waits:
        _split_multi_waits(nc)
    return nc


def _split_multi_waits(nc):
    """walrus (this build) accepts at most one sync-wait per instruction;
    merge same-semaphore waits to their max threshold, then hoist any
    remaining extra waits onto standalone same-engine EventSemaphore ops."""
    for blk in nc.main_func.blocks:
        out = []
        for ins in blk.instructions:
            si = ins.sync_info
            waits = list(si.on_wait) if si is not None else []
            if len(waits) > 1:
                merged = {}
                for w in waits:
                    key = (w.sync_type, w.id)
                    prev = merged.get(key)
                    if (prev is None or (w.wait_value or 0) >
                            (prev.wait_value or 0)):
                        merged[key] = w
                waits = list(merged.values())
                if len(waits) == 1:
                    ins.sync_info = type(si)(on_wait=waits,
                                             on_update=list(si.on_update))
            if len(waits) > 1:
                for k, w in enumerate(waits[:-1]):
                    ev = mybir.InstEventSemaphore(
                        name=f"{ins.name}w{k}", ins=[], outs=[])
                    ev.engine = ins.engine
                    ev.sync_info = type(si)(on_wait=[w], on_update=[])
                    out.append(ev)
                ins.sync_info = type(si)(on_wait=[waits[-1]],
                                         on_update=list(si.on_update))
            out.append(ins)
        blk.instructions = out
    return nc


def _body(ctx, tc, x_bm, x_tr, w1t, wmw, bias3, u_out, std4, mean4, reps):
    nc = tc.nc

    const = ctx.enter_context(tc.tile_pool(name="const", bufs=1))
    xtp = ctx.enter_context(tc.tile_pool(name="xtp", bufs=1))
    hp = ctx.enter_context(tc.tile_pool(name="hp", bufs=4))
    mp = ctx.enter_context(tc.tile_pool(name="mp", bufs=6))
    hs = ctx.enter_context(tc.tile_pool(name="hs", bufs=1))
    qp = ctx.enter_context(tc.tile_pool(name="qp", bufs=1))
    # PSUM: h pairs [128,1024] x2 bufs = 4 banks; m [128,512] x2 = 2;
    # two head accumulators (one per half) = 2  -> exactly 8 banks
    ps_h = ctx.enter_context(tc.tile_pool(name="ps_h", bufs=2, space="PSUM"))
    ps_m = ctx.enter_context(tc.tile_pool(name="ps_m", bufs=2, space="PSUM"))
    ps_hd = ctx.enter_context(tc.tile_pool(name="ps_hd", bufs=1, space="PSUM"))

    # warmup scratch: memset so the PE burst has no DMA dependency
    wscr = const.tile([32, CH], MMDT, name="wscr", tag="wscr")
    nc.gpsimd.memset(wscr, 0.0)

    # ---- constants / weights on the gpsimd ring so the sync ring's
    # first transfer is the x_tr slice the first L1 matmul needs ----
    w1g_sb = const.tile([128, 128], MMDT)   # W1eff^T in 4 row groups
    for g in range(4):
        nc.gpsimd.dma_start(out=w1g_sb[32 * g:32 * g + 8, :], in_=w1t[:, :])
    wmw_sb = const.tile([128, 208], MMDT)
    nc.gpsimd.dma_start(out=wmw_sb, in_=wmw[:, :])
    wmt_sb = wmw_sb[:, 0:128]
    wz_sb = wmw_sb[:, 128:208]     # heads at col 15+16v of this window
    bias3_sb = const.tile([128, 3], F32)
    nc.gpsimd.dma_start(out=bias3_sb, in_=bias3[:, :])
    b1_sb = bias3_sb[:, 0:1]
    bm_sb = bias3_sb[:, 1:2]
    bh_sb = bias3_sb[:, 2:3]

    for _ in range(reps):
        _body_rep(nc, tc, const, xtp, hp, mp, hs, qp, ps_h, ps_m, ps_hd,
                  x_bm, x_tr, u_out, w1g_sb, wmt_sb, wz_sb, b1_sb, bm_sb,
                  bh_sb, std4, mean4, wscr)


def _qp_pre(nc, qp, half, x_sb, std4, mean4):
    """x-only dCBF terms for one half. Host precomputed per-row fp16:
    slot0=dx, 1=dy, 2=wrapped sin phase, 3=v, 4=wrapped cos phase,
    5=8*barrier, 6=v^2. Device: 2 ScalarE sins + contiguous GpSimd
    tensor_tensor chain. Reciprocal is traced later via pre_recip.
    Tiles are [128, 128] batch-major: col s*64+c is quarter-set s's
    row p*64+c."""
    pe = nc.gpsimd

    def t(name, w=128):
        nm = f"{name}_{half}"
        return qp.tile([128, w], F32, name=nm, tag=nm)

    def xf(i):  # contiguous [128, 128] fp16 feature slice
        return x_sb[:, 128 * i:128 * (i + 1)]

    ST, CT = t("ST"), t("CT")
    nc.scalar.activation(ST, xf(2), AF.Sin)
    nc.scalar.activation(CT, xf(4), AF.Sin)

    # VBA pair tile: [VB | Aq] per quarter-set: [128, (s, comp, 64)]
    vba = qp.tile([128, 256], F32, name=f"vba_{half}", tag=f"vba_{half}")
    vba4 = vba[:].rearrange("p (s comp c) -> p s comp c", s=2, comp=2)
    VBv = vba4[:, :, 0, :]      # [128, 2, 64]
    Av = vba4[:, :, 1, :]

    def v3(tl):
        return tl[:].rearrange("p (s c) -> p s c", s=2)

    t1, t2, t3, t4, Bq = t("t1"), t("t2"), t("t3"), t("t4"), t("Bq")
    pe.tensor_tensor(t1, xf(0), CT, ALU.mult)
    pe.tensor_tensor(t2, xf(1), ST, ALU.mult)
    pe.tensor_tensor(Av, v3(t1), v3(t2), ALU.add)        # A = dx ct + dy st
    pe.tensor_tensor(t3, xf(0), ST, ALU.mult)
    pe.tensor_tensor(t4, xf(1), CT, ALU.mult)
    pe.tensor_tensor(Bq, t3, t4, ALU.subtract)   # B = dx st - dy ct
    pe.tensor_tensor(VBv, v3(Bq), xf(3).rearrange("p (s c) -> p s c", s=2),
                     ALU.mult)                           # VB (G1 = 2 VB)

    VA4n, VB2, A2, GGn = t("VA4n"), t("VB2"), t("A2"), t("GGn")
    pe.tensor_tensor(v3(VA4n), Av,
                     xf(3).rearrange("p (s c) -> p s c", s=2), ALU.mult)
    pe.tensor_tensor(v3(VB2), VBv, VBv, ALU.mult)        # VB^2
    pe.tensor_tensor(v3(A2), Av, Av, ALU.mult)           # A^2
    pe.tensor_tensor(GGn, VB2, A2, ALU.add)
    R = t("R")
    return dict(vba=vba, VA4n=VA4n, GGn=GGn, R=R, x=x_sb)


def _qp_pre_recip(nc, pre):
    """VectorE tail of qp_pre (scalar folds + reciprocal). Traced late,
    under a tile_wait_until floor, so the scheduler never queues it ahead
    of bulk VectorE work while GpSimd is still producing GGn."""
    nc.vector.tensor_scalar(pre["VA4n"], pre["VA4n"], -4.0, None, ALU.mult)
    nc.vector.tensor_scalar(pre["GGn"], pre["GGn"], -1.0, -2.5e-13,
                            ALU.mult, ALU.subtract)   # -(VB^2+A^2+eps)
    nc.vector.reciprocal(pre["R"], pre["GGn"])   # -1/(VB^2+A^2+eps)


def _qp_post(nc, qp, half, set_, pre, hsb, u_out, ve, ve2=None):
    """Head-dependent QP tail for one quarter (half, set_). hsb holds the
    drained heads: partition 16v+j (j = within-set chunk), +64 for set B.
    With ve2, every elementwise op is emitted twice on column halves
    (ve gets cols [0:32], ve2 [32:64] of each HQ block) to halve the
    serial chain latency in the kernel tail."""
    q = half * 2 + set_

    def t(name, w=HQ):
        nm = f"{name}_q{q}"
        return qp.tile([128, w], F32, name=nm, tag=nm)

    sl = slice(set_ * HQ, (set_ + 1) * HQ)
    vba_q = pre["vba"][:, set_ * 2 * HQ:(set_ + 1) * 2 * HQ]  # [VB | Aq]
    VA4n_q, R_q = pre["VA4n"][:, sl], pre["R"][:, sl]
    BAR8_q = pre["x"][:, 5 * 128 + set_ * HQ:5 * 128 + (set_ + 1) * HQ]
    VSQ_q = pre["x"][:, 6 * 128 + set_ * HQ:6 * 128 + (set_ + 1) * HQ]

    # reshape heads into batch-major quarter tiles: PN = [p1n | p2n],
    # SG = [sg1 | sg2]; p = j*8 + w//64, col = w%64  (r_q = p*64 + c)
    PN = t("PN", 2 * HQ)
    SG = t("SG", 2 * HQ)
    pb = 64 * set_
    for v, (dst, c0) in enumerate([(PN, 0), (PN, HQ), (SG, 0), (SG, HQ)]):
        eng = nc.sync if v % 2 == 0 else nc.gpsimd
        eng.dma_start(
            out=dst[:, c0:c0 + HQ],
            in_=hsb[pb + 16 * v:pb + 16 * v + 16, :].rearrange(
                "j (q c) -> j q c", q=8),
        )

    if ve2 is None:
        splits = [(ve, 0, HQ)]
    else:
        splits = [(ve, 0, HQ // 2), (ve2, HQ // 2, HQ)]

    SS, SP, T5p8, T4dh = t("SS"), t("SP"), t("T5p8"), t("T4dh")
    TP = t("TP", 2 * HQ)
    T3h, q2h, NUM2 = t("T3h"), t("q2h"), t("NUM2")
    L0, LAM2 = t("L0"), t("LAM2")
    M12 = t("M12", 2 * HQ)
    U = t("U", 2 * HQ)
    uv = U[:].rearrange("p (c v) -> p v c", v=2)
    for e, a, b in splits:
        s = slice(a, b)
        s2 = slice(HQ + a, HQ + b)
        e.tensor_tensor(SS[:, s], SG[:, s], SG[:, s2], ALU.add)
        e.tensor_tensor(SP[:, s], SG[:, s], SG[:, s2], ALU.mult)
        e.tensor_tensor(T5p8[:, s], BAR8_q[:, s], SP[:, s], ALU.mult)
        e.tensor_tensor(T4dh[:, s], SS[:, s], VA4n_q[:, s], ALU.mult)
        e.tensor_tensor(TP[:, s], vba_q[:, s], PN[:, s], ALU.mult)
        e.tensor_tensor(TP[:, s2], vba_q[:, s2], PN[:, s2], ALU.mult)
        e.tensor_tensor(T3h[:, s], TP[:, s], TP[:, s2], ALU.subtract)
        e.tensor_tensor(q2h[:, s], T3h[:, s], VSQ_q[:, s], ALU.subtract)
        e.tensor_tensor(q2h[:, s], q2h[:, s], T4dh[:, s], ALU.add)
        e.tensor_tensor(NUM2[:, s], T5p8[:, s], q2h[:, s], ALU.subtract)
        e.tensor_tensor(L0[:, s], NUM2[:, s], R_q[:, s], ALU.mult)
        e.tensor_scalar(LAM2[:, s], L0[:, s], 0.0, None, ALU.max)
        e.tensor_tensor(M12[:, s], LAM2[:, s], vba_q[:, s], ALU.mult)
        e.tensor_tensor(M12[:, s2], LAM2[:, s], vba_q[:, s2], ALU.mult)
        e.tensor_tensor(uv[:, 0, a:b], PN[:, s], M12[:, s], ALU.subtract)
        e.tensor_tensor(uv[:, 1, a:b], PN[:, s2], M12[:, s2], ALU.add)

    for gl in range(2):
        base = (2 * half + gl) * GSZ + set_ * (QB // 2)
        eng = nc.sync if gl == 0 else nc.gpsimd
        eng.dma_start(
            out=u_out[base:base + QB // 2, :].rearrange(
                "(p c) v -> p (c v)", p=64),
            in_=U[64 * gl:64 * gl + 64, :],
        )


def _body_rep(nc, tc, const, xtp, hp, mp, hs, qp, ps_h, ps_m, ps_hd,
              x_bm, x_tr, u_out, w1g_sb, wmt_sb, wz_sb, b1_sb, bm_sb,
              bh_sb, std4, mean4, wscr):
    # ---- head accumulators; also the PE-warmup dump target ----
    head_ps = [ps_hd.tile([128, CH], F32, name=f"head{h}", tag=f"head{h}")
               for h in range(2)]

    # PE warmup on the memset scratch: keeps the PE busy/ramping while
    # input DMAs run. Overwritten by the A/B sets' start=True later.
    for w in range(N_WARMUP_MM):
        nc.tensor.matmul(head_ps[0], wscr[0:8, 0:128], wscr[0:8, :],
                         start=True, stop=True, tile_position=(0, 0))

    # ---- x loads. x_tr sliced on the sync queue in need-order; the
    # batch-major QP x (feature-blocked fp16) on the gpsimd queue so the
    # two DMA rings run in parallel. ----
    xt_sb = xtp.tile([128, GSZ], MMDT, name="xt_sb", tag="xt_sb")
    slices = [(0, 0, 1024), (1, 0, 1024), (0, 1024, 2048), (1, 1024, 2048),
              (0, 2048, 4096), (1, 2048, 4096),
              (2, 0, 4096), (3, 0, 4096), (0, 4096, 8192), (1, 4096, 8192),
              (2, 4096, 8192), (3, 4096, 8192)]
    for g, c0, c1 in slices:
        nc.sync.dma_start(
            out=xt_sb[32 * g:32 * g + 8, c0:c1],
            in_=x_tr[8 * g:8 * g + 8, c0:c1])
    # xh[p, (f, s, c)]: feature-blocked, host-prepared in exactly this
    # layout so each half is one contiguous 256KB DMA
    x_half = []
    for h in range(2):
        xh = xtp.tile([128, 2 * HQ * NF], FP16, name=f"x_sb{h}",
                      tag=f"x_sb{h}")
        x_half.append(xh)
        nc.gpsimd.dma_start(out=xh, in_=x_bm[h * 128:(h + 1) * 128, :])

    qp_pre = [None, None]

    # ---- MLP pipeline: 32 units (half, sp, gl), software-pipelined on
    # the PE with a 2-unit lag so L2 never waits on the relu and L3
    # never waits on the m copy:
    #   iteration u: L1x2(u) -> L2x2(u-1) -> L3x2(u-2)
    l2i = 0
    hsb = [hs.tile([128, CH], F32, name=f"hsb{h}", tag=f"hsb{h}")
           for h in range(2)]
    h_sb_q = {}
    m_sb_q = {}

    def unit(u):
        half, r = u // 16, u % 16
        return half, r // 2, r % 2   # (half, sp, gl)

    for u in range(34):
        if u < 32:
            half, sp, gl = unit(u)
            g = 2 * half + gl
            h_ps = ps_h.tile([128, 2 * CH], F32, name="h_ps", tag="h_ps")
            for k in range(2):
                nc.tensor.matmul(
                    h_ps[:, k * CH:(k + 1) * CH],
                    w1g_sb[32 * g:32 * g + 8, :],
                    xt_sb[32 * g:32 * g + 8,
                          (2 * sp + k) * CH:(2 * sp + k + 1) * CH],
                    start=True, stop=True,
                    tile_position=(32 * g, 0),
                )
            h_sb = hp.tile([128, 2 * CH], MMDT, name="h_sb", tag="h_sb")
            nc.scalar.activation(h_sb, h_ps, AF.Relu, bias=b1_sb,
                                 scale=1.0)
            h_sb_q[u] = h_sb

        if 0 <= u - 1 < 32:
            v = u - 1
            h_sb = h_sb_q.pop(v)
            ms = []
            for k in range(2):
                m_ps = ps_m.tile([128, CH], F32, name="m_ps", tag="m_ps")
                nc.tensor.matmul(
                    m_ps, wmt_sb, h_sb[:, k * CH:(k + 1) * CH],
                    start=True, stop=True)
                m_sb = mp.tile([128, CH], MMDT, name="m_sb", tag="m_sb")
                l2i += 1
                if L2_ACT_EVERY and l2i % L2_ACT_EVERY == 0:
                    nc.scalar.activation(m_sb, m_ps, AF.Relu,
                                         bias=bm_sb, scale=1.0)
                else:
                    nc.vector.tensor_scalar(m_sb, m_ps, bm_sb, 0.0,
                                            ALU.add, ALU.max)
                ms.append(m_sb)
            m_sb_q[v] = ms

        if 0 <= u - 2 < 32:
            w = u - 2
            halfw, spw, glw = unit(w)
            set_w = spw // 4
            splw = spw % 4
            ms = m_sb_q.pop(w)
            for k in range(2):
                jA = glw * 8 + 2 * splw + k   # within-set chunk index
                first = (splw == 0 and glw == 0 and k == 0)
                last = (splw == 3 and glw == 1 and k == 1)
                nc.tensor.matmul(
                    head_ps[halfw][64 * set_w:64 * set_w + 64, :],
                    wz_sb[:, 15 - jA:79 - jA],
                    ms[k],
                    start=first, stop=last,
                    tile_position=(0, 64 * set_w),
                )
            if last:
                # this quarter's 16 chunks are accumulated: drain
                # (p-heads negated on VectorE, pen sigmoid on ScalarE)
                # and run its QP tail
                pb = 64 * set_w
                nc.vector.tensor_scalar(
                    hsb[halfw][pb:pb + 32, :],
                    head_ps[halfw][pb:pb + 32, :],
                    bh_sb[pb:pb + 32, :], -1.0, ALU.add, ALU.mult)
                nc.scalar.activation(
                    hsb[halfw][pb + 32:pb + 64, :],
                    head_ps[halfw][pb + 32:pb + 64, :],
                    AF.Sigmoid, bias=bh_sb[pb + 32:pb + 64, :], scale=1.0)
                if halfw == 1 and set_w == 1:
                    _qp_post(nc, qp, halfw, set_w, qp_pre[halfw],
                             hsb[halfw], u_out, nc.vector, nc.gpsimd)
                else:
                    _qp_post(nc, qp, halfw, set_w, qp_pre[halfw],
                             hsb[halfw], u_out, nc.gpsimd)

        if u == 2:
            # both halves' x-only QP prefixes: GpSimd chain + the two
            # ScalarE sins per half (adjacent -> one act-table swap).
            # Floored so the sins/table-loads never displace the first
            # L1 relus on ScalarE.
            with tc.tile_wait_until(0.014):
                qp_pre[0] = _qp_pre(nc, qp, 0, x_half[0], std4, mean4)
                qp_pre[1] = _qp_pre(nc, qp, 1, x_half[1], std4, mean4)
        if u == 7:
            with tc.tile_wait_until(0.024):
                _qp_pre_recip(nc, qp_pre[0])
        if u == 11:
            with tc.tile_wait_until(0.034):
                _qp_pre_recip(nc, qp_pre[1])


def _host_prepare(inputs):
    """Fold mean/std into L1, build packed weight/bias tensors."""
    import ml_dtypes

    x = np.ascontiguousarray(inputs["x"], dtype=np.float32)
    mean = np.asarray(inputs["mean"], dtype=np.float32)
    std = np.asarray(inputs["std"], dtype=np.float32)
    W1 = np.asarray(inputs["W1"], dtype=np.float32)
    b1 = np.asarray(inputs["b1"], dtype=np.float32)
    W21 = np.asarray(inputs["W21"], dtype=np.float32)
    b21 = np.asarray(inputs["b21"], dtype=np.float32)
    W22 = np.asarray(inputs["W22"], dtype=np.float32)
    b22 = np.asarray(inputs["b22"], dtype=np.float32)
    W31 = np.asarray(inputs["W31"], dtype=np.float32)
    b31 = np.asarray(inputs["b31"], dtype=np.float32)
    W32 = np.asarray(inputs["W32"], dtype=np.float32)
    b32 = np.asarray(inputs["b32"], dtype=np.float32)

    W1eff = W1 * std[None, :]                      # [128, 8]
    b1eff = (b1 + W1 @ mean).astype(np.float32)    # [128]
    w1t = np.ascontiguousarray(W1eff.T)            # [8, 128]

    Wmid = np.vstack([W21, W22]).astype(np.float32)   # [128, 128]
    wmt = np.ascontiguousarray(Wmid.T)
    bmid = np.concatenate([b21, b22]).astype(np.float32)[:, None]

    # head weights: v=0,1 -> W31 (mid cols 0:64), v=2,3 -> W32 (64:128);
    # window tile [128, 80] with head v at col 15+16v
    Whead = np.zeros((4, 128), np.float32)
    Whead[0:2, 0:64] = W31
    Whead[2:4, 64:128] = W32
    wz = np.zeros((128, 80), np.float32)
    for v in range(4):
        wz[:, 15 + 16 * v] = Whead[v, :]

    # head biases, [128]: partition 64*set + 16v + j. The p-head drain
    # computes -(head + bias) so store +b31 there; sigmoid drain adds.
    bh64 = np.zeros(64, np.float32)
    bh64[0:16] = b31[0]
    bh64[16:32] = b31[1]
    bh64[32:48] = b32[0]
    bh64[48:64] = b32[1]
    bhead = np.concatenate([bh64, bh64])[:, None]

    std4 = tuple(float(std[i]) for i in range(4))
    mean4 = tuple(float(mean[i]) for i in range(4))

    mmnp = {F32R: np.float32, F32: np.float32,
            BF16: ml_dtypes.bfloat16}[MMDT]
    w1t = w1t.astype(mmnp)
    wmw = np.ascontiguousarray(
        np.concatenate([wmt, wz], axis=1)).astype(mmnp)
    bias3 = np.ascontiguousarray(
        np.concatenate([b1eff[:, None], bmid, bhead], axis=1))

    common = {"w1t": w1t, "wmw": wmw, "bias3": bias3}

    # QP-side x features, precomputed in fp32 then cast to fp16:
    # [dx, dy, wrapped sin phase, v, wrapped cos phase, 8*barrier, v^2, 0]
    px = x[:, 0] * std[0] + mean[0]
    py = x[:, 1] * std[1] + mean[1]
    th = x[:, 2] * std[2] + mean[2]
    vv = x[:, 3] * std[3] + mean[3]
    dx, dy = px - OBS_X, py - OBS_Y
    tau = 2.0 * np.pi

    def wrap(a):
        return a - tau * np.round(a / tau)

    feats = np.stack([
        dx, dy, wrap(th), vv, wrap(th + 0.5 * np.pi),
        8.0 * (dx * dx + dy * dy - RAD * RAD), vv * vv,
        np.zeros_like(dx)], axis=1).astype(np.float16)

    in_maps = []
    for c in range(NCORES):
        xs = x[c * BC:(c + 1) * BC]               # [32768, 8]
        # transposed / grouped layout: row 8g+f = feature f of group g
        xtr = np.ascontiguousarray(
            xs.reshape(4, GSZ, NF).transpose(0, 2, 1).reshape(
                32, GSZ)).astype(mmnp)
        # QP x in the on-chip layout: [h, p=gl*64+p', (f, s, c)] where
        # batch row = (2h+gl)*8192 + s*4096 + p'*64 + c
        fc = feats[c * BC:(c + 1) * BC].reshape(2, 2, 2, 64, 64, NF)
        xbm = np.ascontiguousarray(
            fc.transpose(0, 1, 3, 5, 2, 4).reshape(256, 2 * HQ * NF))
        in_maps.append({"x_bm": xbm, "x_tr": xtr, **common})
    return in_maps, std4, mean4


def kernel(**inputs):
    in_maps, std4, mean4 = _host_prepare(inputs)
    nc = _build_program(std4, mean4)
    last_err = None
    for attempt in range(3):
        try:
            res = run_bass_kernel_spmd(nc, in_maps, list(range(NCORES)))
            break
        except Exception as e:  # transient axon/NRT flakes
            last_err = e
            if attempt == 2:
                raise
            import time

            time.sleep(5)
    u = np.concatenate([res.results[c]["u"] for c in range(NCORES)], axis=0)
    return u.astype(np.float32)


if __name__ == "__main__":
    rng = np.random.default_rng(0)
    demo = {
        "x": rng.standard_normal((B, NF), dtype=np.float32),
        "mean": np.zeros(NF, np.float32),
        "std": np.ones(NF, np.float32),
        "W1": rng.standard_normal((128, NF), dtype=np.float32) * 0.3,
        "b1": rng.standard_normal(128, dtype=np.float32) * 0.3,
        "W21": rng.standard_normal((64, 128), dtype=np.float32) * 0.08,
        "b21": rng.standard_normal(64, dtype=np.float32) * 0.08,
        "W22": rng.standard_normal((64, 128), dtype=np.float32) * 0.08,
        "b22": rng.standard_normal(64, dtype=np.float32) * 0.08,
        "W31": rng.standard_normal((2, 64), dtype=np.float32) * 0.1,
        "b31": rng.standard_normal(2, dtype=np.float32) * 0.1,
        "W32": rng.standard_normal((2, 64), dtype=np.float32) * 0.1,
        "b32": rng.standard_normal(2, dtype=np.float32) * 0.1,
        "sgn": np.int64(1),
    }
    out = kernel(**demo)
    print(out.shape, out.dtype)


# revision 24
# speedup vs baseline: 1.1048x; 1.1048x over previous
"""BarrierNet (MLP heads + dCBF closed-form QP) Trainium2 Bass kernel.

Data-parallel over 8 NeuronCores: batch 262144 is split into 8 shards of
32768 rows; the tiny MLP weights are replicated (folded with mean/std on
host) and each core computes its full shard independently. No collectives.

Per-core dataflow (feature-major matmuls, batch in the free dim, all
matmuls bf16 in / fp32 PSUM out; PE-bound):
  L1: h^T = relu(W1eff @ x^T + b1eff); two adjacent 512-col chunks of
      one group per unit, same stationary, into a [128, 1024] PSUM pair
      drained by one ScalarE activation.
  L2: mid^T = relu(Wmid @ h^T + bmid), drained chunk-wise by VectorE
      (every L2_ACT_EVERY-th by ScalarE for balance).
  L3: heads via a sliding-window weight trick, split into an A set
      (first 16 chunks of each half -> PSUM partitions 0:64) and a B set
      (last 16 -> 64:128) so each quarter of the batch drains as soon as
      its 16 chunks are accumulated (tail shrinks to one quarter's QP).
  The PE stream is software-pipelined with a 2-unit lag
  (L1(u) -> L2(u-1) -> L3(u-2)) so no matmul waits on a PSUM drain.
  QP: the host precomputes [dx, dy, wrapped sin/cos phases, 8*barrier,
      v^2] in fp16 (feature-blocked, SBUF layout built host-side), so
      qp_pre is just 2 ScalarE sins + a contiguous GpSimd
      tensor_tensor chain per half; the scalar folds + reciprocal run
      on VectorE under a tile_wait_until floor (the Tile scheduler's
      cost model underestimates GpSimd, and without the floor it queues
      them ahead of the bulk m copies, head-of-line blocking VectorE
      and starving the PE). qp_post runs per quarter on GpSimd; the
      final quarter is column-split across VectorE+GpSimd to halve the
      tail latency.

Matmul perf notes: 512 cols is the ISA max per matmul; fp32r executes
as fp32 HIGH (4x slower) on this walrus, bf16 streams 1 col/cycle.
The HAM clock duty-cycles between full and half rate under sustained
all-core load, so the sustained floor is ~(512 + ldw) cycles at
1.2 GHz per matmul; fp8 DoubleRow for L3 measured 2.0e-2 error (at the
gate) and was rejected.
"""

import os
import sys

import numpy as np

sys.path.insert(0, "/opt/trn_rl_repo")

import concourse.bass as bass
import concourse.tile as tile
from concourse import mybir
from concourse.bass_utils import run_bass_kernel_spmd

F32 = mybir.dt.float32
F32R = mybir.dt.float32r
BF16 = mybir.dt.bfloat16
FP16 = mybir.dt.float16
AF = mybir.ActivationFunctionType
ALU = mybir.AluOpType

B = 262144
NF = 8
NCORES = 8
BC = B // NCORES   # 32768 rows per core
CH = 512           # chunk columns (one PSUM bank of fp32)
GSZ = 8192         # rows per group (4 groups per core)
HB = 16384         # rows per half
QB = 8192          # rows per quarter
HQ = 64            # quarter batch-major free width (r_q = p*64 + c)
OBS_X, OBS_Y, RAD = 4.0, 6.0, 1.5
PI = float(np.pi)

N_WARMUP_MM = int(os.environ.get("KERNEL_WARMUP", "0"))
# every Nth L2 copy goes to ScalarE instead of VectorE (0 = none)
L2_ACT_EVERY = int(os.environ.get("KERNEL_L2_ACT_EVERY", "3"))

_MMDT_MAP = {"f32r": F32R, "f32": F32, "bf16": BF16}
MMDT = _MMDT_MAP[os.environ.get("KERNEL_MM_DTYPE", "bf16")]


def _build_program(std4, mean4, split_waits=True, reps=1):
    nc = bass.Bass()

    x_bm = nc.dram_tensor("x_bm", [256, 2 * HQ * NF], FP16,
                          kind="ExternalInput")
    x_tr = nc.dram_tensor("x_tr", [32, GSZ], MMDT, kind="ExternalInput")
    w1t = nc.dram_tensor("w1t", [8, 128], MMDT, kind="ExternalInput")
    wmw = nc.dram_tensor("wmw", [128, 208], MMDT, kind="ExternalInput")
    bias3 = nc.dram_tensor("bias3", [128, 3], F32, kind="ExternalInput")
    u_out = nc.dram_tensor("u", [BC, 2], F32, kind="ExternalOutput")

    with tile.TileContext(nc) as tc:
        from contextlib import ExitStack

        with ExitStack() as ctx:
            _body(ctx, tc, x_bm, x_tr, w1t, wmw, bias3, u_out,
                  std4, mean4, reps)
    if split_waits:
        _split_multi_waits(nc)
    return nc


def _split_multi_waits(nc):
    """walrus (this build) accepts at most one sync-wait per instruction;
    merge same-semaphore waits to their max threshold, then hoist any
    remaining extra waits onto standalone same-engine EventSemaphore ops."""
    for blk in nc.main_func.blocks:
        out = []
        for ins in blk.instructions:
            si = ins.sync_info
            waits = list(si.on_wait) if si is not None else []
            if len(waits) > 1:
                merged = {}
                for w in waits:
                    key = (w.sync_type, w.id)
                    prev = merged.get(key)
                    if (prev is None or (w.wait_value or 0) >
                            (prev.wait_value or 0)):
                        merged[key] = w
                waits = list(merged.values())
                if len(waits) == 1:
                    ins.sync_info = type(si)(on_wait=waits,
                                             on_update=list(si.on_update))
            if len(waits) > 1:
                for k, w in enumerate(waits[:-1]):
                    ev = mybir.InstEventSemaphore(
                        name=f"{ins.name}w{k}", ins=[], outs=[])
                    ev.engine = ins.engine
                    ev.sync_info = type(si)(on_wait=[w], on_update=[])
                    out.append(ev)
                ins.sync_info = type(si)(on_wait=[waits[-1]],
                                         on_update=list(si.on_update))
            out.append(ins)
        blk.instructions = out
    return nc


def _body(ctx, tc, x_bm, x_tr, w1t, wmw, bias3, u_out, std4, mean4, reps):
    nc = tc.nc

    const = ctx.enter_context(tc.tile_pool(name="const", bufs=1))
    xtp = ctx.enter_context(tc.tile_pool(name="xtp", bufs=1))
    hp = ctx.enter_context(tc.tile_pool(name="hp", bufs=4))
    mp = ctx.enter_context(tc.tile_pool(name="mp", bufs=6))
    hs = ctx.enter_context(tc.tile_pool(name="hs", bufs=1))
    qp = ctx.enter_context(tc.tile_pool(name="qp", bufs=1))
    # PSUM: h pairs [128,1024] x2 bufs = 4 banks; m [128,512] x2 = 2;
    # two head accumulators (one per half) = 2  -> exactly 8 banks
    ps_h = ctx.enter_context(tc.tile_pool(name="ps_h", bufs=2, space="PSUM"))
    ps_m = ctx.enter_context(tc.tile_pool(name="ps_m", bufs=2, space="PSUM"))
    ps_hd = ctx.enter_context(tc.tile_pool(name="ps_hd", bufs=1, space="PSUM"))

    # warmup scratch: memset so the PE burst has no DMA dependency
    wscr = const.tile([32, CH], MMDT, name="wscr", tag="wscr")
    nc.gpsimd.memset(wscr, 0.0)

    # ---- constants / weights on the gpsimd ring so the sync ring's
    # first transfer is the x_tr slice the first L1 matmul needs ----
    w1g_sb = const.tile([128, 128], MMDT)   # W1eff^T in 4 row groups
    for g in range(4):
        nc.gpsimd.dma_start(out=w1g_sb[32 * g:32 * g + 8, :], in_=w1t[:, :])
    wmw_sb = const.tile([128, 208], MMDT)
    nc.gpsimd.dma_start(out=wmw_sb, in_=wmw[:, :])
    wmt_sb = wmw_sb[:, 0:128]
    wz_sb = wmw_sb[:, 128:208]     # heads at col 15+16v of this window
    bias3_sb = const.tile([128, 3], F32)
    nc.gpsimd.dma_start(out=bias3_sb, in_=bias3[:, :])
    b1_sb = bias3_sb[:, 0:1]
    bm_sb = bias3_sb[:, 1:2]
    bh_sb = bias3_sb[:, 2:3]

    for _ in range(reps):
        _body_rep(nc, tc, const, xtp, hp, mp, hs, qp, ps_h, ps_m, ps_hd,
                  x_bm, x_tr, u_out, w1g_sb, wmt_sb, wz_sb, b1_sb, bm_sb,
                  bh_sb, std4, mean4, wscr)


def _qp_pre(nc, qp, half, x_sb, std4, mean4):
    """x-only dCBF terms for one half. Host precomputed per-row fp16:
    slot0=dx, 1=dy, 2=wrapped sin phase, 3=v, 4=wrapped cos phase,
    5=8*barrier, 6=v^2. Device: 2 ScalarE sins + contiguous GpSimd
    tensor_tensor chain. Reciprocal is traced later via pre_recip.
    Tiles are [128, 128] batch-major: col s*64+c is quarter-set s's
    row p*64+c."""
    pe = nc.gpsimd

    def t(name, w=128):
        nm = f"{name}_{half}"
        return qp.tile([128, w], F32, name=nm, tag=nm)

    def xf(i):  # contiguous [128, 128] fp16 feature slice
        return x_sb[:, 128 * i:128 * (i + 1)]

    ST, CT = t("ST"), t("CT")
    nc.scalar.activation(ST, xf(2), AF.Sin)
    nc.scalar.activation(CT, xf(4), AF.Sin)

    # VBA pair tile: [VB | Aq] per quarter-set: [128, (s, comp, 64)]
    vba = qp.tile([128, 256], F32, name=f"vba_{half}", tag=f"vba_{half}")
    vba4 = vba[:].rearrange("p (s comp c) -> p s comp c", s=2, comp=2)
    VBv = vba4[:, :, 0, :]      # [128, 2, 64]
    Av = vba4[:, :, 1, :]

    def v3(tl):
        return tl[:].rearrange("p (s c) -> p s c", s=2)

    t1, t2, t3, t4, Bq = t("t1"), t("t2"), t("t3"), t("t4"), t("Bq")
    pe.tensor_tensor(t1, xf(0), CT, ALU.mult)
    pe.tensor_tensor(t2, xf(1), ST, ALU.mult)
    pe.tensor_tensor(Av, v3(t1), v3(t2), ALU.add)        # A = dx ct + dy st
    pe.tensor_tensor(t3, xf(0), ST, ALU.mult)
    pe.tensor_tensor(t4, xf(1), CT, ALU.mult)
    pe.tensor_tensor(Bq, t3, t4, ALU.subtract)   # B = dx st - dy ct
    pe.tensor_tensor(VBv, v3(Bq), xf(3).rearrange("p (s c) -> p s c", s=2),
                     ALU.mult)                           # VB (G1 = 2 VB)

    VA4n, VB2, A2, GGn = t("VA4n"), t("VB2"), t("A2"), t("GGn")
    pe.tensor_tensor(v3(VA4n), Av,
                     xf(3).rearrange("p (s c) -> p s c", s=2), ALU.mult)
    pe.tensor_tensor(v3(VB2), VBv, VBv, ALU.mult)        # VB^2
    pe.tensor_tensor(v3(A2), Av, Av, ALU.mult)           # A^2
    pe.tensor_tensor(GGn, VB2, A2, ALU.add)
    R = t("R")
    return dict(vba=vba, VA4n=VA4n, GGn=GGn, R=R, x=x_sb)


def _qp_pre_recip(nc, pre):
    """VectorE tail of qp_pre (scalar folds + reciprocal). Traced late,
    under a tile_wait_until floor, so the scheduler never queues it ahead
    of bulk VectorE work while GpSimd is still producing GGn."""
    nc.vector.tensor_scalar(pre["VA4n"], pre["VA4n"], -4.0, None, ALU.mult)
    nc.vector.tensor_scalar(pre["GGn"], pre["GGn"], -1.0, -2.5e-13,
                            ALU.mult, ALU.subtract)   # -(VB^2+A^2+eps)
    nc.vector.reciprocal(pre["R"], pre["GGn"])   # -1/(VB^2+A^2+eps)


def _qp_post(nc, qp, half, set_, pre, hsb, u_out, ve, ve2=None):
    """Head-dependent QP tail for one quarter (half, set_). hsb holds the
    drained heads: partition 16v+j (j = within-set chunk), +64 for set B.
    With ve2, every elementwise op is emitted twice on column halves
    (ve gets cols [0:32], ve2 [32:64] of each HQ block) to halve the
    serial chain latency in the kernel tail."""
    q = half * 2 + set_

    def t(name, w=HQ):
        nm = f"{name}_q{q}"
        return qp.tile([128, w], F32, name=nm, tag=nm)

    sl = slice(set_ * HQ, (set_ + 1) * HQ)
    vba_q = pre["vba"][:, set_ * 2 * HQ:(set_ + 1) * 2 * HQ]  # [VB | Aq]
    VA4n_q, R_q = pre["VA4n"][:, sl], pre["R"][:, sl]
    BAR8_q = pre["x"][:, 5 * 128 + set_ * HQ:5 * 128 + (set_ + 1) * HQ]
    VSQ_q = pre["x"][:, 6 * 128 + set_ * HQ:6 * 128 + (set_ + 1) * HQ]

    # reshape heads into batch-major quarter tiles: PN = [p1n | p2n],
    # SG = [sg1 | sg2]; p = j*8 + w//64, col = w%64  (r_q = p*64 + c)
    PN = t("PN", 2 * HQ)
    SG = t("SG", 2 * HQ)
    pb = 64 * set_
    rings = [nc.sync, nc.gpsimd, nc.scalar, nc.sync]
    for v, (dst, c0) in enumerate([(PN, 0), (PN, HQ), (SG, 0), (SG, HQ)]):
        eng = rings[v]
        eng.dma_start(
            out=dst[:, c0:c0 + HQ],
            in_=hsb[pb + 16 * v:pb + 16 * v + 16, :].rearrange(
                "j (q c) -> j q c", q=8),
        )

    if ve2 is None:
        splits = [(ve, 0, HQ)]
    else:
        splits = [(ve, 0, HQ // 2), (ve2, HQ // 2, HQ)]

    SS, SP, T5p8, T4dh = t("SS"), t("SP"), t("T5p8"), t("T4dh")
    TP = t("TP", 2 * HQ)
    T3h, q2h, NUM2 = t("T3h"), t("q2h"), t("NUM2")
    L0, LAM2 = t("L0"), t("LAM2")
    M12 = t("M12", 2 * HQ)
    U = t("U", 2 * HQ)
    uv = U[:].rearrange("p (c v) -> p v c", v=2)
    for e, a, b in splits:
        s = slice(a, b)
        s2 = slice(HQ + a, HQ + b)
        e.tensor_tensor(SS[:, s], SG[:, s], SG[:, s2], ALU.add)
        e.tensor_tensor(SP[:, s], SG[:, s], SG[:, s2], ALU.mult)
        e.tensor_tensor(T5p8[:, s], BAR8_q[:, s], SP[:, s], ALU.mult)
        e.tensor_tensor(T4dh[:, s], SS[:, s], VA4n_q[:, s], ALU.mult)
        e.tensor_tensor(TP[:, s], vba_q[:, s], PN[:, s], ALU.mult)
        e.tensor_tensor(TP[:, s2], vba_q[:, s2], PN[:, s2], ALU.mult)
        e.tensor_tensor(T3h[:, s], TP[:, s], TP[:, s2], ALU.subtract)
        e.tensor_tensor(q2h[:, s], T3h[:, s], VSQ_q[:, s], ALU.subtract)
        e.tensor_tensor(q2h[:, s], q2h[:, s], T4dh[:, s], ALU.add)
        e.tensor_tensor(NUM2[:, s], T5p8[:, s], q2h[:, s], ALU.subtract)
        e.tensor_tensor(L0[:, s], NUM2[:, s], R_q[:, s], ALU.mult)
        e.tensor_scalar(LAM2[:, s], L0[:, s], 0.0, None, ALU.max)
        e.tensor_tensor(M12[:, s], LAM2[:, s], vba_q[:, s], ALU.mult)
        e.tensor_tensor(M12[:, s2], LAM2[:, s], vba_q[:, s2], ALU.mult)
        e.tensor_tensor(uv[:, 0, a:b], PN[:, s], M12[:, s], ALU.subtract)
        e.tensor_tensor(uv[:, 1, a:b], PN[:, s2], M12[:, s2], ALU.add)

    for gl in range(2):
        base = (2 * half + gl) * GSZ + set_ * (QB // 2)
        if ve2 is None:
            eng = nc.sync if gl == 0 else nc.gpsimd
            eng.dma_start(
                out=u_out[base:base + QB // 2, :].rearrange(
                    "(p c) v -> p (c v)", p=64),
                in_=U[64 * gl:64 * gl + 64, :],
            )
        else:
            for ci, (c0, c1) in enumerate([(0, HQ // 2), (HQ // 2, HQ)]):
                eng = (nc.sync, nc.gpsimd)[(gl + ci) % 2]
                eng.dma_start(
                    out=u_out[base:base + QB // 2, :].rearrange(
                        "(p c) v -> p c v", p=64)[:, c0:c1, :],
                    in_=U[64 * gl:64 * gl + 64, 2 * c0:2 * c1].rearrange(
                        "p (c v) -> p c v", v=2),
                )


def _body_rep(nc, tc, const, xtp, hp, mp, hs, qp, ps_h, ps_m, ps_hd,
              x_bm, x_tr, u_out, w1g_sb, wmt_sb, wz_sb, b1_sb, bm_sb,
              bh_sb, std4, mean4, wscr):
    # ---- head accumulators; also the PE-warmup dump target ----
    head_ps = [ps_hd.tile([128, CH], F32, name=f"head{h}", tag=f"head{h}")
               for h in range(2)]

    # PE warmup on the memset scratch: keeps the PE busy/ramping while
    # input DMAs run. Overwritten by the A/B sets' start=True later.
    for w in range(N_WARMUP_MM):
        nc.tensor.matmul(head_ps[0], wscr[0:8, 0:128], wscr[0:8, :],
                         start=True, stop=True, tile_position=(0, 0))

    # ---- x loads. x_tr sliced on the sync queue in need-order; the
    # batch-major QP x (feature-blocked fp16) on the gpsimd queue so the
    # two DMA rings run in parallel. ----
    xt_sb = xtp.tile([128, GSZ], MMDT, name="xt_sb", tag="xt_sb")
    slices = [(0, 0, 1024), (1, 0, 1024), (0, 1024, 2048), (1, 1024, 2048),
              (0, 2048, 4096), (1, 2048, 4096),
              (2, 0, 4096), (3, 0, 4096), (0, 4096, 8192), (1, 4096, 8192),
              (2, 4096, 8192), (3, 4096, 8192)]
    for g, c0, c1 in slices:
        nc.sync.dma_start(
            out=xt_sb[32 * g:32 * g + 8, c0:c1],
            in_=x_tr[8 * g:8 * g + 8, c0:c1])
    # xh[p, (f, s, c)]: feature-blocked, host-prepared in exactly this
    # layout so each half is one contiguous 256KB DMA
    x_half = []
    for h in range(2):
        xh = xtp.tile([128, 2 * HQ * NF], FP16, name=f"x_sb{h}",
                      tag=f"x_sb{h}")
        x_half.append(xh)
        nc.gpsimd.dma_start(out=xh, in_=x_bm[h * 128:(h + 1) * 128, :])

    qp_pre = [None, None]

    # ---- MLP pipeline: 32 units (half, sp, gl), software-pipelined on
    # the PE with a 2-unit lag so L2 never waits on the relu and L3
    # never waits on the m copy:
    #   iteration u: L1x2(u) -> L2x2(u-1) -> L3x2(u-2)
    l2i = 0
    hsb = [hs.tile([128, CH], F32, name=f"hsb{h}", tag=f"hsb{h}")
           for h in range(2)]
    h_sb_q = {}
    m_sb_q = {}

    def unit(u):
        half, r = u // 16, u % 16
        return half, r // 2, r % 2   # (half, sp, gl)

    for u in range(34):
        if u < 32:
            half, sp, gl = unit(u)
            g = 2 * half + gl
            h_ps = ps_h.tile([128, 2 * CH], F32, name="h_ps", tag="h_ps")
            for k in range(2):
                nc.tensor.matmul(
                    h_ps[:, k * CH:(k + 1) * CH],
                    w1g_sb[32 * g:32 * g + 8, :],
                    xt_sb[32 * g:32 * g + 8,
                          (2 * sp + k) * CH:(2 * sp + k + 1) * CH],
                    start=True, stop=True,
                    tile_position=(32 * g, 0),
                )
            h_sb = hp.tile([128, 2 * CH], MMDT, name="h_sb", tag="h_sb")
            nc.scalar.activation(h_sb, h_ps, AF.Relu, bias=b1_sb,
                                 scale=1.0)
            h_sb_q[u] = h_sb

        if 0 <= u - 1 < 32:
            v = u - 1
            h_sb = h_sb_q.pop(v)
            ms = []
            for k in range(2):
                m_ps = ps_m.tile([128, CH], F32, name="m_ps", tag="m_ps")
                nc.tensor.matmul(
                    m_ps, wmt_sb, h_sb[:, k * CH:(k + 1) * CH],
                    start=True, stop=True)
                m_sb = mp.tile([128, CH], MMDT, name="m_sb", tag="m_sb")
                l2i += 1
                if L2_ACT_EVERY and l2i % L2_ACT_EVERY == 0:
                    nc.scalar.activation(m_sb, m_ps, AF.Relu,
                                         bias=bm_sb, scale=1.0)
                else:
                    nc.vector.tensor_scalar(m_sb, m_ps, bm_sb, 0.0,
                                            ALU.add, ALU.max)
                ms.append(m_sb)
            m_sb_q[v] = ms

        if 0 <= u - 2 < 32:
            w = u - 2
            halfw, spw, glw = unit(w)
            set_w = spw // 4
            splw = spw % 4
            ms = m_sb_q.pop(w)
            for k in range(2):
                jA = glw * 8 + 2 * splw + k   # within-set chunk index
                first = (splw == 0 and glw == 0 and k == 0)
                last = (splw == 3 and glw == 1 and k == 1)
                nc.tensor.matmul(
                    head_ps[halfw][64 * set_w:64 * set_w + 64, :],
                    wz_sb[:, 15 - jA:79 - jA],
                    ms[k],
                    start=first, stop=last,
                    tile_position=(0, 64 * set_w),
                )
            if last:
                # this quarter's 16 chunks are accumulated: drain
                # (p-heads negated on VectorE, pen sigmoid on ScalarE)
                # and run its QP tail
                pb = 64 * set_w
                nc.vector.tensor_scalar(
                    hsb[halfw][pb:pb + 32, :],
                    head_ps[halfw][pb:pb + 32, :],
                    bh_sb[pb:pb + 32, :], -1.0, ALU.add, ALU.mult)
                nc.scalar.activation(
                    hsb[halfw][pb + 32:pb + 64, :],
                    head_ps[halfw][pb + 32:pb + 64, :],
                    AF.Sigmoid, bias=bh_sb[pb + 32:pb + 64, :], scale=1.0)
                if halfw == 1 and set_w == 1:
                    _qp_post(nc, qp, halfw, set_w, qp_pre[halfw],
                             hsb[halfw], u_out, nc.vector, nc.gpsimd)
                else:
                    _qp_post(nc, qp, halfw, set_w, qp_pre[halfw],
                             hsb[halfw], u_out, nc.gpsimd)

        if u == 2:
            # both halves' x-only QP prefixes: GpSimd chain + the two
            # ScalarE sins per half (adjacent -> one act-table swap).
            # Floored so the sins/table-loads never displace the first
            # L1 relus on ScalarE.
            with tc.tile_wait_until(0.014):
                qp_pre[0] = _qp_pre(nc, qp, 0, x_half[0], std4, mean4)
                qp_pre[1] = _qp_pre(nc, qp, 1, x_half[1], std4, mean4)
        if u == 7:
            with tc.tile_wait_until(0.024):
                _qp_pre_recip(nc, qp_pre[0])
        if u == 11:
            with tc.tile_wait_until(0.034):
                _qp_pre_recip(nc, qp_pre[1])


def _host_prepare(inputs):
    """Fold mean/std into L1, build packed weight/bias tensors."""
    import ml_dtypes

    x = np.ascontiguousarray(inputs["x"], dtype=np.float32)
    mean = np.asarray(inputs["mean"], dtype=np.float32)
    std = np.asarray(inputs["std"], dtype=np.float32)
    W1 = np.asarray(inputs["W1"], dtype=np.float32)
    b1 = np.asarray(inputs["b1"], dtype=np.float32)
    W21 = np.asarray(inputs["W21"], dtype=np.float32)
    b21 = np.asarray(inputs["b21"], dtype=np.float32)
    W22 = np.asarray(inputs["W22"], dtype=np.float32)
    b22 = np.asarray(inputs["b22"], dtype=np.float32)
    W31 = np.asarray(inputs["W31"], dtype=np.float32)
    b31 = np.asarray(inputs["b31"], dtype=np.float32)
    W32 = np.asarray(inputs["W32"], dtype=np.float32)
    b32 = np.asarray(inputs["b32"], dtype=np.float32)

    W1eff = W1 * std[None, :]                      # [128, 8]
    b1eff = (b1 + W1 @ mean).astype(np.float32)    # [128]
    w1t = np.ascontiguousarray(W1eff.T)            # [8, 128]

    Wmid = np.vstack([W21, W22]).astype(np.float32)   # [128, 128]
    wmt = np.ascontiguousarray(Wmid.T)
    bmid = np.concatenate([b21, b22]).astype(np.float32)[:, None]

    # head weights: v=0,1 -> W31 (mid cols 0:64), v=2,3 -> W32 (64:128);
    # window tile [128, 80] with head v at col 15+16v
    Whead = np.zeros((4, 128), np.float32)
    Whead[0:2, 0:64] = W31
    Whead[2:4, 64:128] = W32
    wz = np.zeros((128, 80), np.float32)
    for v in range(4):
        wz[:, 15 + 16 * v] = Whead[v, :]

    # head biases, [128]: partition 64*set + 16v + j. The p-head drain
    # computes -(head + bias) so store +b31 there; sigmoid drain adds.
    bh64 = np.zeros(64, np.float32)
    bh64[0:16] = b31[0]
    bh64[16:32] = b31[1]
    bh64[32:48] = b32[0]
    bh64[48:64] = b32[1]
    bhead = np.concatenate([bh64, bh64])[:, None]

    std4 = tuple(float(std[i]) for i in range(4))
    mean4 = tuple(float(mean[i]) for i in range(4))

    mmnp = {F32R: np.float32, F32: np.float32,
            BF16: ml_dtypes.bfloat16}[MMDT]
    w1t = w1t.astype(mmnp)
    wmw = np.ascontiguousarray(
        np.concatenate([wmt, wz], axis=1)).astype(mmnp)
    bias3 = np.ascontiguousarray(
        np.concatenate([b1eff[:, None], bmid, bhead], axis=1))

    common = {"w1t": w1t, "wmw": wmw, "bias3": bias3}

    # QP-side x features, precomputed in fp32 then cast to fp16:
    # [dx, dy, wrapped sin phase, v, wrapped cos phase, 8*barrier, v^2, 0]
    px = x[:, 0] * std[0] + mean[0]
    py = x[:, 1] * std[1] + mean[1]
    th = x[:, 2] * std[2] + mean[2]
    vv = x[:, 3] * std[3] + mean[3]
    dx, dy = px - OBS_X, py - OBS_Y
    tau = 2.0 * np.pi

    def wrap(a):
        return a - tau * np.round(a / tau)

    feats = np.stack([
        dx, dy, wrap(th), vv, wrap(th + 0.5 * np.pi),
        8.0 * (dx * dx + dy * dy - RAD * RAD), vv * vv,
        np.zeros_like(dx)], axis=1).astype(np.float16)

    in_maps = []
    for c in range(NCORES):
        xs = x[c * BC:(c + 1) * BC]               # [32768, 8]
        # transposed / grouped layout: row 8g+f = feature f of group g
        xtr = np.ascontiguousarray(
            xs.reshape(4, GSZ, NF).transpose(0, 2, 1).reshape(
                32, GSZ)).astype(mmnp)
        # QP x in the on-chip layout: [h, p=gl*64+p', (f, s, c)] where
        # batch row = (2h+gl)*8192 + s*4096 + p'*64 + c
        fc = feats[c * BC:(c + 1) * BC].reshape(2, 2, 2, 64, 64, NF)
        xbm = np.ascontiguousarray(
            fc.transpose(0, 1, 3, 5, 2, 4).reshape(256, 2 * HQ * NF))
        in_maps.append({"x_bm": xbm, "x_tr": xtr, **common})
    return in_maps, std4, mean4


def kernel(**inputs):
    in_maps, std4, mean4 = _host_prepare(inputs)
    nc = _build_program(std4, mean4)
    last_err = None
    for attempt in range(3):
        try:
            res = run_bass_kernel_spmd(nc, in_maps, list(range(NCORES)))
            break
        except Exception as e:  # transient axon/NRT flakes
            last_err = e
            if attempt == 2:
                raise
            import time

            time.sleep(5)
    u = np.concatenate([res.results[c]["u"] for c in range(NCORES)], axis=0)
    return u.astype(np.float32)


if __name__ == "__main__":
    rng = np.random.default_rng(0)
    demo = {
        "x": rng.standard_normal((B, NF), dtype=np.float32),
        "mean": np.zeros(NF, np.float32),
        "std": np.ones(NF, np.float32),
        "W1": rng.standard_normal((128, NF), dtype=np.float32) * 0.3,
        "b1": rng.standard_normal(128, dtype=np.float32) * 0.3,
        "W21": rng.standard_normal((64, 128), dtype=np.float32) * 0.08,
        "b21": rng.standard_normal(64, dtype=np.float32) * 0.08,
        "W22": rng.standard_normal((64, 128), dtype=np.float32) * 0.08,
        "b22": rng.standard_normal(64, dtype=np.float32) * 0.08,
        "W31": rng.standard_normal((2, 64), dtype=np.float32) * 0.1,
        "b31": rng.standard_normal(2, dtype=np.float32) * 0.1,
        "W32": rng.standard_normal((2, 64), dtype=np.float32) * 0.1,
        "b32": rng.standard_normal(2, dtype=np.float32) * 0.1,
        "sgn": np.int64(1),
    }
    out = kernel(**demo)
    print(out.shape, out.dtype)
